# revision 7
# baseline (speedup 1.0000x reference)
"""DimeNet-like GNN (DLMPNN) forward on 8 Trainium2 NeuronCores (SPMD).

Host: sort/pad edges by (core, idx_i) into fixed 512-wide supers aligned to
512-atom windows; sort/pad triplets by (core, target-edge window) with a
fixed (max-over-cores) number of 512-triplet supers per edge window. All
aggregations become mask-matmuls accumulated in PSUM over fixed windows.
Device: feature-major bf16 activations, weights-stationary matmuls, one
AllToAll (acting as AllGather) per interaction block to exchange x_kj.
"""
import numpy as np
import ml_dtypes

E = 128
NR = 6
NABF = 7
NB = 3
CUTOFF = 5.0
NCORES = 8
AW = 512          # atoms per atom-window
EW = 512          # edges per edge-super / edge-window
TW = 512          # triplets per triplet-super

bf = ml_dtypes.bfloat16


def _cheb(c, n):
    out = np.empty((n,) + c.shape, np.float32)
    out[0] = 1.0
    out[1] = c
    for k in range(2, n):
        out[k] = 2.0 * c * out[k - 1] - out[k - 2]
    return out


def _prep(inp):
    A = inp["Z"].shape[0]
    batch_seg = np.asarray(inp["batch_seg"]).astype(np.int64)
    M = int(batch_seg.max()) + 1
    idx_i = np.asarray(inp["idx_i"]).astype(np.int64)
    idx_j = np.asarray(inp["idx_j"]).astype(np.int64)
    idx_kj = np.asarray(inp["idx_kj"]).astype(np.int64)
    idx_ji = np.asarray(inp["idx_ji"]).astype(np.int64)
    R = np.asarray(inp["R"]).astype(np.float32)
    Z = np.asarray(inp["Z"]).astype(np.int64)
    Eg = idx_i.shape[0]

    # ---- atom ranges at molecule boundaries
    mol_first = np.searchsorted(batch_seg, np.arange(M))
    a_b, m_b = [0], [0]
    for c in range(1, NCORES):
        tgt = c * A // NCORES
        m = int(np.searchsorted(mol_first, tgt))
        if m > 0 and (m == M or mol_first[m] - tgt > tgt - mol_first[m - 1]):
            m -= 1
        m = max(m_b[-1] + 1, min(m, M - (NCORES - c)))
        m_b.append(m)
        a_b.append(int(mol_first[m]))
    a_b.append(A)
    m_b.append(M)
    a_lo, a_hi = np.array(a_b[:-1]), np.array(a_b[1:])
    m_lo, m_hi = np.array(m_b[:-1]), np.array(m_b[1:])
    assert (m_hi - m_lo).max() <= 128
    core_of_atom = np.zeros(A, np.int64)
    for c in range(NCORES):
        core_of_atom[a_lo[c]:a_hi[c]] = c

    NAW = int(-(-int((a_hi - a_lo).max()) // AW))
    PA_ = NAW * AW

    # ---- edge scheduling: per (core, atom window) lists, padded to S_aw
    e_owner = core_of_atom[idx_i]
    ail_all = idx_i - a_lo[e_owner]          # local atom id
    e_aw = ail_all // AW                     # atom window per edge
    # count per (core, aw)
    cnt = np.zeros((NCORES, NAW), np.int64)
    for c in range(NCORES):
        sel = e_owner == c
        cnt[c] = np.bincount(e_aw[sel], minlength=NAW)
    S_aw = np.maximum(1, -(-cnt.max(axis=0) // EW))      # supers per window
    aw_super0 = np.concatenate([[0], np.cumsum(S_aw)])   # first super of aw
    NES = int(aw_super0[-1])                             # edge supers total
    PE_ = NES * EW

    # slot assignment
    slot_of_edge = np.full(Eg, -1, np.int64)
    for c in range(NCORES):
        for aw in range(NAW):
            sel = np.where((e_owner == c) & (e_aw == aw))[0]
            sel = sel[np.argsort(ail_all[sel], kind="stable")]
            base = aw_super0[aw] * EW
            slot_of_edge[sel] = base + np.arange(len(sel))
    # global padded slot (for gather table, split in halves for A2A)
    H = PE_ // 2

    def gslot(owner, pos):
        half = (pos >= H).astype(np.int64)
        return half * (NCORES * H) + owner * H + (pos - half * H)

    # ---- per-core edge arrays
    d = np.linalg.norm(R[idx_i] - R[idx_j], axis=-1)
    d = np.maximum(d, 1e-2)
    freq = np.arange(1, NR + 1, dtype=np.float32) * np.pi
    rbf_all = (np.sqrt(np.float32(2.0 / CUTOFF))
               * np.sin(freq[None, :] * (d / CUTOFF)[:, None])
               / d[:, None]).astype(np.float32)
    x0 = np.asarray(inp["emb_atom"]).astype(np.float32)[Z]

    rbfT = np.zeros((NCORES, NR, PE_), np.float32)
    x0iT = np.zeros((NCORES, E, PE_), np.float32)
    x0jT = np.zeros((NCORES, E, PE_), np.float32)
    asid = np.full((NCORES, 128, PE_ // 128), -1.0, np.float32)
    for c in range(NCORES):
        sel = np.where(e_owner == c)[0]
        pos = slot_of_edge[sel]
        rbfT[c, :, pos] = rbf_all[sel]
        x0iT[c][:, pos] = x0[idx_i[sel]].T
        x0jT[c][:, pos] = x0[idx_j[sel]].T
        # asid: relative to the atom window of the super the edge sits in
        aw = e_aw[sel]
        relv = (ail_all[sel] - aw * AW).astype(np.float32)
        p = pos % 128
        col = pos // 128
        asid[c, p, col] = relv

    # ---- triplet scheduling per (core, edge window)
    NEW_ = NES                                  # edge windows == edge supers
    t_owner = e_owner[idx_ji]
    t_tpos = slot_of_edge[idx_ji]
    t_w = t_tpos // EW
    tcnt = np.zeros((NCORES, NEW_), np.int64)
    for c in range(NCORES):
        sel = t_owner == c
        tcnt[c] = np.bincount(t_w[sel], minlength=NEW_)
    S_tw = np.maximum(1, -(-tcnt.max(axis=0) // TW))
    tw_super0 = np.concatenate([[0], np.cumsum(S_tw)])
    NTS = int(tw_super0[-1])
    PT_ = NTS * TW

    cosc = np.clip(np.asarray(inp["cosine_ijk"]).astype(np.float32), -1.0, 1.0)
    abf_all = _cheb(cosc, NABF)

    gidx = np.zeros((NCORES, 128, PT_ // 128), np.int32)
    tsid = np.full((NCORES, 128, PT_ // 128), -1.0, np.float32)
    abfT = np.zeros((NCORES, NABF, PT_), np.float32)
    for c in range(NCORES):
        selc = np.where(t_owner == c)[0]
        for w in np.unique(t_w[selc]):
            sel = selc[t_w[selc] == w]
            sel = sel[np.argsort(t_tpos[sel], kind="stable")]
            tpos = tw_super0[w] * TW + np.arange(len(sel))
            abfT[c][:, tpos] = abf_all[:, sel]
            gidx[c, tpos % 128, tpos // 128] = gslot(
                e_owner[idx_kj[sel]],
                slot_of_edge[idx_kj[sel]]).astype(np.int32)
            tsid[c, tpos % 128, tpos // 128] = (
                t_tpos[sel] - w * EW).astype(np.float32)

    # ---- atom-level arrays
    s_init = np.zeros((NCORES, E, PA_), np.float32)
    msid = np.full((NCORES, 128, PA_ // 128), -1.0, np.float32)
    for c in range(NCORES):
        na = int(a_hi[c] - a_lo[c])
        s_init[c, :, :na] = x0[a_lo[c]:a_hi[c]].T
        mloc = (batch_seg[a_lo[c]:a_hi[c]] - m_lo[c]).astype(np.float32)
        ar = np.arange(na)
        msid[c, ar % 128, ar // 128] = mloc

    # ---- weights / biases packs
    wnames, wmats = [], []

    def addw(name, mat):
        w = np.zeros((128, 128), np.float32)
        w[:mat.shape[0], :] = mat
        wnames.append(name)
        wmats.append(w)

    W_emb = np.asarray(inp["W_emb"]).astype(np.float32)
    addw("emb0", W_emb[0:128]); addw("emb1", W_emb[128:256])
    addw("emb2", W_emb[256:384])
    addw("rbf_emb", np.asarray(inp["W_rbf_emb"]).astype(np.float32))
    for i in range(NB + 1):
        addw(f"orbf{i}", np.asarray(inp["W_orbf"]).astype(np.float32)[i])
        addw(f"o1_{i}", np.asarray(inp["W_o1"]).astype(np.float32)[i])
        addw(f"out{i}", np.asarray(inp["W_out"]).astype(np.float32)[i])
        addw(f"id1_{i}", np.asarray(inp["W_id1"]).astype(np.float32)[i])
        addw(f"id2_{i}", np.asarray(inp["W_id2"]).astype(np.float32)[i])
    for i in range(NB):
        addw(f"ji{i}", np.asarray(inp["W_ji"]).astype(np.float32)[i])
        addw(f"kj{i}", np.asarray(inp["W_kj"]).astype(np.float32)[i])
        addw(f"irbf{i}", np.asarray(inp["W_irbf"]).astype(np.float32)[i])
        addw(f"abf{i}", np.asarray(inp["W_abf"]).astype(np.float32)[i])
        addw(f"res{i}", np.asarray(inp["W_res"]).astype(np.float32)[i])
    wpack = np.concatenate(wmats, axis=1).astype(bf)
    wslot = {n: i for i, n in enumerate(wnames)}

    bnames, bvecs = [], []

    def addb(name, vec):
        bnames.append(name)
        bvecs.append(np.asarray(vec).astype(np.float32).reshape(E, 1))

    addb("emb", inp["b_emb"])
    for i in range(NB + 1):
        addb(f"o1_{i}", np.asarray(inp["b_o1"])[i])
        addb(f"id1_{i}", np.asarray(inp["b_id1"])[i])
    for i in range(NB):
        addb(f"ji{i}", np.asarray(inp["b_ji"])[i])
        addb(f"kj{i}", np.asarray(inp["b_kj"])[i])
        addb(f"res{i}", np.asarray(inp["b_res"])[i])
    bpack = np.concatenate(bvecs, axis=1)
    bslot = {n: i for i, n in enumerate(bnames)}

    iota = np.tile(np.arange(512, dtype=np.float32), (128, 1))
    ident = np.eye(128, dtype=np.float32).astype(bf)

    in_maps = []
    for c in range(NCORES):
        in_maps.append({
            "rbfT": rbfT[c].astype(bf), "x0iT": x0iT[c].astype(bf),
            "x0jT": x0jT[c].astype(bf), "asid": asid[c],
            "gidx": gidx[c], "tsid": tsid[c], "abfT": abfT[c].astype(bf),
            "s_init": s_init[c], "msid": msid[c],
            "wpack": wpack, "bpack": bpack, "iota": iota, "ident": ident,
        })
    meta = dict(PE=PE_, PT=PT_, PA=PA_, NAW=NAW, NES=NES, NTS=NTS,
                S_aw=S_aw.tolist(), aw_super0=aw_super0.tolist(),
                S_tw=S_tw.tolist(), tw_super0=tw_super0.tolist(),
                m_lo=m_lo, m_hi=m_hi, M=M,
                coef_mp=float(np.asarray(inp["coef_mp"]).reshape(-1)[0]),
                coef_sg=float(np.asarray(inp["coef_sg"]).reshape(-1)[0]),
                nw=len(wnames), wslot=wslot, nb=len(bnames), bslot=bslot)
    return in_maps, meta


# ------------------------------------------------------------------ device

def _build(meta, use_ag=False):
    from concourse import bass, bacc, mybir, tile
    nc = bacc.Bacc("TRN2", target_bir_lowering=False, debug=False,
                   num_devices=NCORES)
    f32 = mybir.dt.float32
    bf16 = mybir.dt.bfloat16
    i32 = mybir.dt.int32
    SILU = mybir.ActivationFunctionType.Silu
    EQ = mybir.AluOpType.is_equal
    MULT = mybir.AluOpType.mult
    ADD = mybir.AluOpType.add
    PE_, PT_, PA_ = meta["PE"], meta["PT"], meta["PA"]
    NAW, NES, NTS = meta["NAW"], meta["NES"], meta["NTS"]
    S_aw, aw_super0 = meta["S_aw"], meta["aw_super0"]
    S_tw, tw_super0 = meta["S_tw"], meta["tw_super0"]
    NW, NBI = meta["nw"], meta["nb"]
    wslot, bslot = meta["wslot"], meta["bslot"]
    H = PE_ // 2

    din = {}
    for name, shape, dt in [
        ("rbfT", [NR, PE_], bf16), ("x0iT", [E, PE_], bf16),
        ("x0jT", [E, PE_], bf16), ("asid", [128, PE_ // 128], f32),
        ("gidx", [128, PT_ // 128], i32), ("tsid", [128, PT_ // 128], f32),
        ("abfT", [NABF, PT_], bf16), ("s_init", [E, PA_], f32),
        ("msid", [128, PA_ // 128], f32), ("wpack", [128, NW * 128], bf16),
        ("bpack", [128, NBI], f32), ("iota", [128, 512], f32),
        ("ident", [128, 128], bf16),
    ]:
        din[name] = nc.dram_tensor(name, shape, dt, kind="ExternalInput")
    dout = nc.dram_tensor("out_mol", [128, E], f32, kind="ExternalOutput")

    x_cur = nc.dram_tensor("x_cur", [E, PE_], bf16, kind="Internal")
    x_ji_h = nc.dram_tensor("x_ji_h", [E, PE_], bf16, kind="Internal")
    xkj_loc = nc.dram_tensor("xkj_loc", [PE_, E], bf16, kind="Internal")
    a2a_in = nc.dram_tensor("a2a_in", [NCORES * H, E], bf16, kind="Internal")
    xkj_full = nc.dram_tensor("xkj_full", [2 * NCORES * H, E], bf16,
                              kind="Internal")

    with tile.TileContext(nc) as tc:
        with tc.tile_pool(name="res", bufs=1) as rp, \
             tc.tile_pool(name="wk", bufs=3) as wk, \
             tc.tile_pool(name="gt", bufs=6) as gp, \
             tc.tile_pool(name="pw", bufs=2, space="PSUM") as pw, \
             tc.tile_pool(name="pa", bufs=1, space="PSUM") as pa, \
             tc.tile_pool(name="pm", bufs=2, space="PSUM") as pm, \
             tc.tile_pool(name="px", bufs=2, space="PSUM") as px, \
             tc.tile_pool(name="ptr", bufs=1, space="PSUM") as ptr_p:

            wpack = rp.tile([128, NW * 128], bf16)
            nc.sync.dma_start(out=wpack[:], in_=din["wpack"][:])
            bpack = rp.tile([128, NBI], f32)
            nc.sync.dma_start(out=bpack[:], in_=din["bpack"][:])
            iota = rp.tile([128, 512], f32)
            nc.sync.dma_start(out=iota[:], in_=din["iota"][:])
            ident = rp.tile([128, 128], bf16)
            nc.sync.dma_start(out=ident[:], in_=din["ident"][:])
            asid = rp.tile([128, PE_ // 128], f32)
            nc.sync.dma_start(out=asid[:], in_=din["asid"][:])
            gidx = rp.tile([128, PT_ // 128], i32)
            nc.sync.dma_start(out=gidx[:], in_=din["gidx"][:])
            tsid = rp.tile([128, PT_ // 128], f32)
            nc.sync.dma_start(out=tsid[:], in_=din["tsid"][:])
            msid = rp.tile([128, PA_ // 128], f32)
            nc.sync.dma_start(out=msid[:], in_=din["msid"][:])
            per_atom = rp.tile([128, PA_], f32)
            res_out = rp.tile([128, PA_], f32)
            s_t = rp.tile([128, PA_], f32)
            nc.sync.dma_start(out=s_t[:], in_=din["s_init"][:])
            nc.vector.memset(res_out[:], 0.0)

            def W(n):
                k = wslot[n]
                return wpack[:, k * 128:(k + 1) * 128]

            def Wk(n, K):
                k = wslot[n]
                return wpack[:, k * 128:(k + 1) * 128][0:K, :]

            def Bv(n):
                return bpack[:, bslot[n]:bslot[n] + 1]

            # -------- helper: transpose 4 sub-tiles of a [128,512] bf16
            def transp4(src):
                rows = []
                for cq in range(4):
                    pt_ = ptr_p.tile([128, 128], bf16, tag="tp")
                    nc.tensor.transpose(out=pt_[:], in_=src[:, cq * 128:(cq + 1) * 128],
                                        identity=ident[:])
                    row = wk.tile([128, 128], bf16, tag="tr")
                    nc.vector.tensor_copy(out=row[:], in_=pt_[:])
                    rows.append(row)
                return rows

            # -------- per-atom aggregation for one edge super
            def atom_agg(g_t, s):
                aw = None
                # find atom window of super s
                for a in range(NAW):
                    if aw_super0[a] <= s < aw_super0[a + 1]:
                        aw = a
                        break
                first = (s == aw_super0[aw])
                last = (s == aw_super0[aw + 1] - 1)
                rows = transp4(g_t)
                if first:
                    pa_t = pa.tile([128, 512], f32, tag="pa")
                    atom_agg.cur = pa_t
                pa_t = atom_agg.cur
                for cq in range(4):
                    mk = wk.tile([128, 512], bf16, tag="amask")
                    nc.vector.tensor_tensor(
                        out=mk[:],
                        in0=asid[:, s * 4 + cq:s * 4 + cq + 1].to_broadcast([128, 512]),
                        in1=iota[:], op=EQ)
                    nc.tensor.matmul(out=pa_t[:], lhsT=rows[cq][:], rhs=mk[:],
                                     start=(first and cq == 0),
                                     stop=(last and cq == 3))
                if last:
                    nc.vector.tensor_tensor(
                        out=per_atom[:, aw * 512:(aw + 1) * 512],
                        in0=per_atom[:, aw * 512:(aw + 1) * 512],
                        in1=pa_t[:], op=ADD)

            # -------- output block tail: per_atom -> res_out
            def ob_tail(i):
                for t in range(PA_ // 512):
                    sl = slice(t * 512, (t + 1) * 512)
                    pa_bf = wk.tile([128, 512], bf16, tag="pabf")
                    nc.vector.tensor_copy(out=pa_bf[:], in_=per_atom[:, sl])
                    p1 = pm.tile([128, 512], f32, tag="pm")
                    nc.tensor.matmul(out=p1[:], lhsT=W(f"o1_{i}"), rhs=pa_bf[:],
                                     start=True, stop=True)
                    h = wk.tile([128, 512], bf16, tag="hob")
                    nc.scalar.activation(out=h[:], in_=p1[:], func=SILU,
                                         bias=Bv(f"o1_{i}"))
                    p2 = pm.tile([128, 512], f32, tag="pm")
                    nc.tensor.matmul(out=p2[:], lhsT=W(f"out{i}"), rhs=h[:],
                                     start=True, stop=True)
                    nc.vector.tensor_tensor(out=res_out[:, sl],
                                            in0=res_out[:, sl], in1=p2[:],
                                            op=ADD)

            def identity_block(i, overwrite=False):
                for t in range(PA_ // 512):
                    sl = slice(t * 512, (t + 1) * 512)
                    s_bf = wk.tile([128, 512], bf16, tag="sbf")
                    nc.vector.tensor_copy(out=s_bf[:], in_=s_t[:, sl])
                    p1 = pm.tile([128, 512], f32, tag="pm")
                    nc.tensor.matmul(out=p1[:], lhsT=W(f"id1_{i}"), rhs=s_bf[:],
                                     start=True, stop=True)
                    h = wk.tile([128, 512], bf16, tag="hid")
                    nc.scalar.activation(out=h[:], in_=p1[:], func=SILU,
                                         bias=Bv(f"id1_{i}"))
                    p2 = pm.tile([128, 512], f32, tag="pm")
                    nc.tensor.matmul(out=p2[:], lhsT=W(f"id2_{i}"), rhs=h[:],
                                     start=True, stop=True)
                    if overwrite:
                        nc.vector.tensor_copy(out=s_t[:, sl], in_=p2[:])
                    else:
                        nc.vector.tensor_tensor(out=s_t[:, sl], in0=s_t[:, sl],
                                                in1=p2[:], op=ADD)

            # ================= embedding + output block 0
            nc.vector.memset(per_atom[:], 0.0)
            for s in range(NES):
                sl = slice(s * 512, (s + 1) * 512)
                xi = wk.tile([128, 512], bf16, tag="xi")
                nc.sync.dma_start(out=xi[:], in_=din["x0iT"][:, sl])
                xj = wk.tile([128, 512], bf16, tag="xj")
                nc.sync.dma_start(out=xj[:], in_=din["x0jT"][:, sl])
                rb = wk.tile([NR, 512], bf16, tag="rb")
                nc.sync.dma_start(out=rb[:], in_=din["rbfT"][:, sl])
                p1 = pm.tile([128, 512], f32, tag="pm")
                nc.tensor.matmul(out=p1[:], lhsT=Wk("rbf_emb", NR), rhs=rb[:],
                                 start=True, stop=True)
                rbe = wk.tile([128, 512], bf16, tag="rbe")
                nc.scalar.activation(out=rbe[:], in_=p1[:], func=SILU)
                p2 = pm.tile([128, 512], f32, tag="pm")
                nc.tensor.matmul(out=p2[:], lhsT=W("emb0"), rhs=xi[:],
                                 start=True, stop=False)
                nc.tensor.matmul(out=p2[:], lhsT=W("emb1"), rhs=xj[:],
                                 start=False, stop=False)
                nc.tensor.matmul(out=p2[:], lhsT=W("emb2"), rhs=rbe[:],
                                 start=False, stop=True)
                x_t = wk.tile([128, 512], bf16, tag="xt")
                nc.scalar.activation(out=x_t[:], in_=p2[:], func=SILU,
                                     bias=Bv("emb"))
                nc.sync.dma_start(out=x_cur[:, sl], in_=x_t[:])
                # output block 0 edge part
                p3 = pm.tile([128, 512], f32, tag="pm")
                nc.tensor.matmul(out=p3[:], lhsT=Wk("orbf0", NR), rhs=rb[:],
                                 start=True, stop=True)
                p3b = wk.tile([128, 512], bf16, tag="p3b")
                nc.scalar.copy(out=p3b[:], in_=p3[:])
                g_t = wk.tile([128, 512], bf16, tag="gt")
                nc.vector.tensor_tensor(out=g_t[:], in0=x_t[:], in1=p3b[:],
                                        op=MULT)
                atom_agg(g_t, s)
            ob_tail(0)
            identity_block(0, overwrite=True)

            # ================= interaction blocks
            for i in range(NB):
                # ---- edge pass A: x_ji, x_kj', stores
                for s in range(NES):
                    sl = slice(s * 512, (s + 1) * 512)
                    x_t = wk.tile([128, 512], bf16, tag="xt")
                    nc.sync.dma_start(out=x_t[:], in_=x_cur[:, sl])
                    rb = wk.tile([NR, 512], bf16, tag="rb")
                    nc.sync.dma_start(out=rb[:], in_=din["rbfT"][:, sl])
                    p1 = pm.tile([128, 512], f32, tag="pm")
                    nc.tensor.matmul(out=p1[:], lhsT=W(f"ji{i}"), rhs=x_t[:],
                                     start=True, stop=True)
                    xji = wk.tile([128, 512], bf16, tag="xji")
                    nc.scalar.activation(out=xji[:], in_=p1[:], func=SILU,
                                         bias=Bv(f"ji{i}"))
                    nc.sync.dma_start(out=x_ji_h[:, sl], in_=xji[:])
                    p2 = pm.tile([128, 512], f32, tag="pm")
                    nc.tensor.matmul(out=p2[:], lhsT=W(f"kj{i}"), rhs=x_t[:],
                                     start=True, stop=True)
                    t1 = wk.tile([128, 512], bf16, tag="t1")
                    nc.scalar.activation(out=t1[:], in_=p2[:], func=SILU,
                                         bias=Bv(f"kj{i}"))
                    p3 = pm.tile([128, 512], f32, tag="pm")
                    nc.tensor.matmul(out=p3[:], lhsT=Wk(f"irbf{i}", NR),
                                     rhs=rb[:], start=True, stop=True)
                    p3b = wk.tile([128, 512], bf16, tag="p3b")
                    nc.scalar.copy(out=p3b[:], in_=p3[:])
                    xkj = wk.tile([128, 512], bf16, tag="xkj")
                    nc.vector.tensor_tensor(out=xkj[:], in0=t1[:], in1=p3b[:],
                                            op=MULT)
                    rows = transp4(xkj)
                    for cq in range(4):
                        e0 = s * 512 + cq * 128
                        nc.sync.dma_start(out=xkj_loc[e0:e0 + 128, :],
                                          in_=rows[cq][:])
                # ---- exchange: A2A-as-allgather, two halves
                for h in range(2):
                    for dcore in range(NCORES):
                        nc.gpsimd.dma_start(
                            out=a2a_in[dcore * H:(dcore + 1) * H, :],
                            in_=xkj_loc[h * H:(h + 1) * H, :])
                    nc.gpsimd.collective_compute(
                        "AllToAll", mybir.AluOpType.bypass,
                        replica_groups=[list(range(NCORES))],
                        ins=[a2a_in[:].opt()],
                        outs=[xkj_full[h * NCORES * H:(h + 1) * NCORES * H, :].opt()],
                    )

                # ---- fused triplet + edge pass B + output block i+1
                nc.vector.memset(per_atom[:], 0.0)
                for w in range(NES):
                    # triplet supers of window w -> PSUM [128, 512]
                    pw_t = pw.tile([128, 512], f32, tag="pw")
                    ns = S_tw[w]
                    s0 = tw_super0[w]
                    for q in range(ns):
                        s = s0 + q
                        ab = wk.tile([NABF, 512], bf16, tag="ab")
                        nc.sync.dma_start(out=ab[:],
                                          in_=din["abfT"][:, s * 512:(s + 1) * 512])
                        g4 = []
                        for cq in range(4):
                            col = s * 4 + cq
                            gg = gp.tile([128, 128], bf16, tag="gg")
                            nc.gpsimd.indirect_dma_start(
                                out=gg[:], out_offset=None,
                                in_=xkj_full[:],
                                in_offset=bass.IndirectOffsetOnAxis(
                                    ap=gidx[:, col:col + 1], axis=0))
                            g4.append(gg)
                        pA = px.tile([128, 512], f32, tag="pA")
                        for cq in range(4):
                            nc.tensor.matmul(
                                out=pA[:, cq * 128:(cq + 1) * 128],
                                lhsT=ab[:, cq * 128:(cq + 1) * 128][0:NABF, :],
                                rhs=W(f"abf{i}")[0:NABF, :],
                                start=True, stop=True)
                        A_bf = wk.tile([128, 512], bf16, tag="Abf")
                        nc.scalar.copy(out=A_bf[:], in_=pA[:])
                        for cq in range(4):
                            m_t = wk.tile([128, 128], bf16, tag="mt")
                            nc.vector.tensor_tensor(
                                out=m_t[:], in0=g4[cq][:],
                                in1=A_bf[:, cq * 128:(cq + 1) * 128], op=MULT)
                            mk = wk.tile([128, 512], bf16, tag="tmask")
                            nc.vector.tensor_tensor(
                                out=mk[:],
                                in0=tsid[:, s * 4 + cq:s * 4 + cq + 1].to_broadcast([128, 512]),
                                in1=iota[:], op=EQ)
                            nc.tensor.matmul(out=pw_t[:], lhsT=m_t[:], rhs=mk[:],
                                             start=(q == 0 and cq == 0),
                                             stop=(q == ns - 1 and cq == 3))
                    # edge pass B on window w
                    sl = slice(w * 512, (w + 1) * 512)
                    xji = wk.tile([128, 512], bf16, tag="xji")
                    nc.sync.dma_start(out=xji[:], in_=x_ji_h[:, sl])
                    pwb = wk.tile([128, 512], bf16, tag="pwb")
                    nc.scalar.copy(out=pwb[:], in_=pw_t[:])
                    h_t = wk.tile([128, 512], bf16, tag="ht")
                    nc.vector.tensor_tensor(out=h_t[:], in0=xji[:], in1=pwb[:],
                                            op=ADD)
                    p1 = pm.tile([128, 512], f32, tag="pm")
                    nc.tensor.matmul(out=p1[:], lhsT=W(f"res{i}"), rhs=h_t[:],
                                     start=True, stop=True)
                    rr = wk.tile([128, 512], bf16, tag="rr")
                    nc.scalar.activation(out=rr[:], in_=p1[:], func=SILU,
                                         bias=Bv(f"res{i}"))
                    x_t = wk.tile([128, 512], bf16, tag="xt")
                    nc.vector.tensor_tensor(out=x_t[:], in0=h_t[:], in1=rr[:],
                                            op=ADD)
                    nc.sync.dma_start(out=x_cur[:, sl], in_=x_t[:])
                    # output block i+1 edge part
                    rb = wk.tile([NR, 512], bf16, tag="rb")
                    nc.sync.dma_start(out=rb[:], in_=din["rbfT"][:, sl])
                    p3 = pm.tile([128, 512], f32, tag="pm")
                    nc.tensor.matmul(out=p3[:], lhsT=Wk(f"orbf{i+1}", NR),
                                     rhs=rb[:], start=True, stop=True)
                    p3b = wk.tile([128, 512], bf16, tag="p3b")
                    nc.scalar.copy(out=p3b[:], in_=p3[:])
                    g_t = wk.tile([128, 512], bf16, tag="gt")
                    nc.vector.tensor_tensor(out=g_t[:], in0=x_t[:], in1=p3b[:],
                                            op=MULT)
                    atom_agg(g_t, w)
                ob_tail(i + 1)
                identity_block(i + 1)

            # ================= final molecule aggregation
            pmol = pa.tile([128, 128], f32, tag="pa")
            nt = PA_ // 128
            for t in range(nt):
                sl = slice(t * 128, (t + 1) * 128)
                tmp = wk.tile([128, 128], f32, tag="tmpf")
                nc.scalar.mul(out=tmp[:], in_=res_out[:, sl],
                              mul=float(meta["coef_mp"]))
                tmp2 = wk.tile([128, 128], f32, tag="tmpf2")
                nc.scalar.mul(out=tmp2[:], in_=s_t[:, sl],
                              mul=float(meta["coef_sg"]))
                totf = wk.tile([128, 128], f32, tag="totf")
                nc.vector.tensor_tensor(out=totf[:], in0=tmp[:], in1=tmp2[:],
                                        op=ADD)
                tot = wk.tile([128, 128], bf16, tag="tot")
                nc.vector.tensor_copy(out=tot[:], in_=totf[:])
                ptr = ptr_p.tile([128, 128], bf16, tag="tp")
                nc.tensor.transpose(out=ptr[:], in_=tot[:], identity=ident[:])
                trow = wk.tile([128, 128], bf16, tag="trow")
                nc.vector.tensor_copy(out=trow[:], in_=ptr[:])
                mk = wk.tile([128, 128], bf16, tag="mmask")
                nc.vector.tensor_tensor(
                    out=mk[:],
                    in0=msid[:, t:t + 1].to_broadcast([128, 128]),
                    in1=iota[:, 0:128], op=EQ)
                nc.tensor.matmul(out=pmol[:], lhsT=mk[:], rhs=trow[:],
                                 start=(t == 0), stop=(t == nt - 1))
            outb = wk.tile([128, 128], f32, tag="outb")
            nc.vector.tensor_copy(out=outb[:], in_=pmol[:])
            nc.sync.dma_start(out=dout[:], in_=outb[:])

    nc.finalize()
    return nc


# ------------------------------------------------------------------ entry

LAST_EXEC_NS = None


def _run_spmd(nc, in_maps, n_reps=4):
    """Execute on 8 cores with device-resident inputs; time repeat runs."""
    import time
    import jax
    from jax.sharding import Mesh, PartitionSpec, NamedSharding
    from jax.experimental.shard_map import shard_map
    from concourse import mybir
    from concourse.bass2jax import (
        _bass_exec_p, install_neuronx_cc_hook, partition_id_tensor)

    install_neuronx_cc_hook()
    partition_name = (nc.partition_id_tensor.name
                      if nc.partition_id_tensor else None)
    in_names, out_names, out_avals, zero_outs = [], [], [], []
    for alloc in nc.m.functions[0].allocations:
        if not isinstance(alloc, mybir.MemoryLocationSet):
            continue
        name = alloc.memorylocations[0].name
        if alloc.kind == "ExternalInput":
            if name != partition_name:
                in_names.append(name)
        elif alloc.kind == "ExternalOutput":
            out_names.append(name)
            shape = tuple(alloc.tensor_shape)
            dtype = mybir.dt.np(alloc.dtype)
            out_avals.append(jax.core.ShapedArray(shape, dtype))
            zero_outs.append(np.zeros(shape, dtype))
    all_in_names = list(in_names) + list(out_names)
    if partition_name is not None:
        all_in_names.append(partition_name)

    def _body(*args):
        operands = list(args)
        if partition_name is not None:
            operands.append(partition_id_tensor())
        outs = _bass_exec_p.bind(
            *operands, out_avals=tuple(out_avals),
            in_names=tuple(all_in_names), out_names=tuple(out_names),
            lowering_input_output_aliases=(),
            sim_require_finite=True, sim_require_nnan=True, nc=nc)
        return tuple(outs)

    devices = jax.devices()[:NCORES]
    mesh = Mesh(np.asarray(devices), ("core",))
    nin = len(in_names) + len(zero_outs)
    fn = jax.jit(shard_map(_body, mesh=mesh,
                           in_specs=(PartitionSpec("core"),) * nin,
                           out_specs=(PartitionSpec("core"),) * len(out_names),
                           check_rep=False), keep_unused=True)
    sharding = NamedSharding(mesh, PartitionSpec("core"))
    args = []
    for name in in_names:
        cat = np.concatenate([np.asarray(in_maps[c][name])
                              for c in range(NCORES)], axis=0)
        args.append(jax.device_put(cat, sharding))
    for z in zero_outs:
        cat = np.zeros((NCORES * z.shape[0], *z.shape[1:]), z.dtype)
        args.append(jax.device_put(cat, sharding))
    jax.block_until_ready(args)

    outs = fn(*args)
    jax.block_until_ready(outs)
    times = []
    for _ in range(n_reps):
        t0 = time.perf_counter()
        o = fn(*args)
        jax.block_until_ready(o)
        times.append(time.perf_counter() - t0)
    global LAST_EXEC_NS
    LAST_EXEC_NS = int(min(times) * 1e9) if times else None
    results = []
    for c in range(NCORES):
        d = {}
        for i, name in enumerate(out_names):
            shape = out_avals[i].shape
            d[name] = np.asarray(outs[i]).reshape(NCORES, *shape)[c]
        results.append(d)
    return results


def kernel(**inputs):
    in_maps, meta = _prep(inputs)
    nc = _build(meta)
    results = _run_spmd(nc, in_maps)
    M = meta["M"]
    out = np.zeros((M, E), np.float32)
    for c in range(NCORES):
        mc = meta["m_hi"][c] - meta["m_lo"][c]
        out[meta["m_lo"][c]:meta["m_hi"][c]] = \
            results[c]["out_mol"][:mc, :]
    return out


# revision 8
# speedup vs baseline: 1.0264x; 1.0264x over previous
"""DimeNet-like GNN (DLMPNN) forward on 8 Trainium2 NeuronCores (SPMD).

Host: sort/pad edges by (core, idx_i) into fixed 512-wide supers aligned to
512-atom windows; sort/pad triplets by (core, target-edge window) with a
fixed (max-over-cores) number of 512-triplet supers per edge window. All
aggregations become mask-matmuls accumulated in PSUM over fixed windows.
Device: feature-major bf16 activations, weights-stationary matmuls, one
AllToAll (acting as AllGather) per interaction block to exchange x_kj.
"""
import numpy as np
import ml_dtypes

E = 128
NR = 6
NABF = 7
NB = 3
CUTOFF = 5.0
NCORES = 8
AW = 512          # atoms per atom-window
EW = 512          # edges per edge-super / edge-window
TW = 512          # triplets per triplet-super

bf = ml_dtypes.bfloat16


def _cheb(c, n):
    out = np.empty((n,) + c.shape, np.float32)
    out[0] = 1.0
    out[1] = c
    for k in range(2, n):
        out[k] = 2.0 * c * out[k - 1] - out[k - 2]
    return out


def _prep(inp):
    A = inp["Z"].shape[0]
    batch_seg = np.asarray(inp["batch_seg"]).astype(np.int64)
    M = int(batch_seg.max()) + 1
    idx_i = np.asarray(inp["idx_i"]).astype(np.int64)
    idx_j = np.asarray(inp["idx_j"]).astype(np.int64)
    idx_kj = np.asarray(inp["idx_kj"]).astype(np.int64)
    idx_ji = np.asarray(inp["idx_ji"]).astype(np.int64)
    R = np.asarray(inp["R"]).astype(np.float32)
    Z = np.asarray(inp["Z"]).astype(np.int64)
    Eg = idx_i.shape[0]

    # ---- atom ranges at molecule boundaries
    mol_first = np.searchsorted(batch_seg, np.arange(M))
    a_b, m_b = [0], [0]
    for c in range(1, NCORES):
        tgt = c * A // NCORES
        m = int(np.searchsorted(mol_first, tgt))
        if m > 0 and (m == M or mol_first[m] - tgt > tgt - mol_first[m - 1]):
            m -= 1
        m = max(m_b[-1] + 1, min(m, M - (NCORES - c)))
        m_b.append(m)
        a_b.append(int(mol_first[m]))
    a_b.append(A)
    m_b.append(M)
    a_lo, a_hi = np.array(a_b[:-1]), np.array(a_b[1:])
    m_lo, m_hi = np.array(m_b[:-1]), np.array(m_b[1:])
    assert (m_hi - m_lo).max() <= 128
    core_of_atom = np.zeros(A, np.int64)
    for c in range(NCORES):
        core_of_atom[a_lo[c]:a_hi[c]] = c

    NAW = int(-(-int((a_hi - a_lo).max()) // AW))
    PA_ = NAW * AW

    # ---- edge scheduling: per (core, atom window) lists, padded to S_aw
    e_owner = core_of_atom[idx_i]
    ail_all = idx_i - a_lo[e_owner]          # local atom id
    e_aw = ail_all // AW                     # atom window per edge
    # count per (core, aw)
    cnt = np.zeros((NCORES, NAW), np.int64)
    for c in range(NCORES):
        sel = e_owner == c
        cnt[c] = np.bincount(e_aw[sel], minlength=NAW)
    S_aw = np.maximum(1, -(-cnt.max(axis=0) // EW))      # supers per window
    aw_super0 = np.concatenate([[0], np.cumsum(S_aw)])   # first super of aw
    NES = int(aw_super0[-1])                             # edge supers total
    PE_ = NES * EW

    # slot assignment
    slot_of_edge = np.full(Eg, -1, np.int64)
    for c in range(NCORES):
        for aw in range(NAW):
            sel = np.where((e_owner == c) & (e_aw == aw))[0]
            sel = sel[np.argsort(ail_all[sel], kind="stable")]
            base = aw_super0[aw] * EW
            slot_of_edge[sel] = base + np.arange(len(sel))
    # global padded slot (for gather table, split in halves for A2A)
    H = PE_ // 2

    def gslot(owner, pos):
        half = (pos >= H).astype(np.int64)
        return half * (NCORES * H) + owner * H + (pos - half * H)

    # ---- per-core edge arrays
    d = np.linalg.norm(R[idx_i] - R[idx_j], axis=-1)
    d = np.maximum(d, 1e-2)
    freq = np.arange(1, NR + 1, dtype=np.float32) * np.pi
    rbf_all = (np.sqrt(np.float32(2.0 / CUTOFF))
               * np.sin(freq[None, :] * (d / CUTOFF)[:, None])
               / d[:, None]).astype(np.float32)
    x0 = np.asarray(inp["emb_atom"]).astype(np.float32)[Z]

    rbfT = np.zeros((NCORES, NR, PE_), np.float32)
    x0iT = np.zeros((NCORES, E, PE_), np.float32)
    x0jT = np.zeros((NCORES, E, PE_), np.float32)
    asid = np.full((NCORES, 128, PE_ // 128), -1.0, np.float32)
    for c in range(NCORES):
        sel = np.where(e_owner == c)[0]
        pos = slot_of_edge[sel]
        rbfT[c, :, pos] = rbf_all[sel]
        x0iT[c][:, pos] = x0[idx_i[sel]].T
        x0jT[c][:, pos] = x0[idx_j[sel]].T
        # asid: relative to the atom window of the super the edge sits in
        aw = e_aw[sel]
        relv = (ail_all[sel] - aw * AW).astype(np.float32)
        p = pos % 128
        col = pos // 128
        asid[c, p, col] = relv

    # ---- triplet scheduling per (core, edge window)
    NEW_ = NES                                  # edge windows == edge supers
    t_owner = e_owner[idx_ji]
    t_tpos = slot_of_edge[idx_ji]
    t_w = t_tpos // EW
    tcnt = np.zeros((NCORES, NEW_), np.int64)
    for c in range(NCORES):
        sel = t_owner == c
        tcnt[c] = np.bincount(t_w[sel], minlength=NEW_)
    S_tw = np.maximum(1, -(-tcnt.max(axis=0) // TW))
    tw_super0 = np.concatenate([[0], np.cumsum(S_tw)])
    NTS = int(tw_super0[-1])
    PT_ = NTS * TW

    cosc = np.clip(np.asarray(inp["cosine_ijk"]).astype(np.float32), -1.0, 1.0)
    abf_all = _cheb(cosc, NABF)

    gidx = np.zeros((NCORES, 128, PT_ // 128), np.int32)
    tsid = np.full((NCORES, 128, PT_ // 128), -1.0, np.float32)
    abfT = np.zeros((NCORES, NABF, PT_), np.float32)
    for c in range(NCORES):
        selc = np.where(t_owner == c)[0]
        for w in np.unique(t_w[selc]):
            sel = selc[t_w[selc] == w]
            sel = sel[np.argsort(t_tpos[sel], kind="stable")]
            tpos = tw_super0[w] * TW + np.arange(len(sel))
            abfT[c][:, tpos] = abf_all[:, sel]
            gidx[c, tpos % 128, tpos // 128] = gslot(
                e_owner[idx_kj[sel]],
                slot_of_edge[idx_kj[sel]]).astype(np.int32)
            tsid[c, tpos % 128, tpos // 128] = (
                t_tpos[sel] - w * EW).astype(np.float32)

    # ---- atom-level arrays
    s_init = np.zeros((NCORES, E, PA_), np.float32)
    msid = np.full((NCORES, 128, PA_ // 128), -1.0, np.float32)
    for c in range(NCORES):
        na = int(a_hi[c] - a_lo[c])
        s_init[c, :, :na] = x0[a_lo[c]:a_hi[c]].T
        mloc = (batch_seg[a_lo[c]:a_hi[c]] - m_lo[c]).astype(np.float32)
        ar = np.arange(na)
        msid[c, ar % 128, ar // 128] = mloc

    # ---- weights / biases packs
    wnames, wmats = [], []

    def addw(name, mat):
        w = np.zeros((128, 128), np.float32)
        w[:mat.shape[0], :] = mat
        wnames.append(name)
        wmats.append(w)

    W_emb = np.asarray(inp["W_emb"]).astype(np.float32)
    addw("emb0", W_emb[0:128]); addw("emb1", W_emb[128:256])
    addw("emb2", W_emb[256:384])
    addw("rbf_emb", np.asarray(inp["W_rbf_emb"]).astype(np.float32))
    for i in range(NB + 1):
        addw(f"orbf{i}", np.asarray(inp["W_orbf"]).astype(np.float32)[i])
        addw(f"o1_{i}", np.asarray(inp["W_o1"]).astype(np.float32)[i])
        addw(f"out{i}", np.asarray(inp["W_out"]).astype(np.float32)[i])
        addw(f"id1_{i}", np.asarray(inp["W_id1"]).astype(np.float32)[i])
        addw(f"id2_{i}", np.asarray(inp["W_id2"]).astype(np.float32)[i])
    for i in range(NB):
        addw(f"ji{i}", np.asarray(inp["W_ji"]).astype(np.float32)[i])
        addw(f"kj{i}", np.asarray(inp["W_kj"]).astype(np.float32)[i])
        addw(f"irbf{i}", np.asarray(inp["W_irbf"]).astype(np.float32)[i])
        addw(f"abf{i}", np.asarray(inp["W_abf"]).astype(np.float32)[i])
        addw(f"res{i}", np.asarray(inp["W_res"]).astype(np.float32)[i])
    wpack = np.concatenate(wmats, axis=1).astype(bf)
    wslot = {n: i for i, n in enumerate(wnames)}

    bnames, bvecs = [], []

    def addb(name, vec):
        bnames.append(name)
        bvecs.append(np.asarray(vec).astype(np.float32).reshape(E, 1))

    addb("emb", inp["b_emb"])
    for i in range(NB + 1):
        addb(f"o1_{i}", np.asarray(inp["b_o1"])[i])
        addb(f"id1_{i}", np.asarray(inp["b_id1"])[i])
    for i in range(NB):
        addb(f"ji{i}", np.asarray(inp["b_ji"])[i])
        addb(f"kj{i}", np.asarray(inp["b_kj"])[i])
        addb(f"res{i}", np.asarray(inp["b_res"])[i])
    bpack = np.concatenate(bvecs, axis=1)
    bslot = {n: i for i, n in enumerate(bnames)}

    iota = np.tile(np.arange(512, dtype=np.float32), (128, 1))
    ident = np.eye(128, dtype=np.float32).astype(bf)

    in_maps = []
    for c in range(NCORES):
        in_maps.append({
            "rbfT": rbfT[c].astype(bf), "x0iT": x0iT[c].astype(bf),
            "x0jT": x0jT[c].astype(bf), "asid": asid[c],
            "gidx": gidx[c], "tsid": tsid[c], "abfT": abfT[c].astype(bf),
            "s_init": s_init[c], "msid": msid[c],
            "wpack": wpack, "bpack": bpack, "iota": iota, "ident": ident,
        })
    meta = dict(PE=PE_, PT=PT_, PA=PA_, NAW=NAW, NES=NES, NTS=NTS,
                S_aw=S_aw.tolist(), aw_super0=aw_super0.tolist(),
                S_tw=S_tw.tolist(), tw_super0=tw_super0.tolist(),
                m_lo=m_lo, m_hi=m_hi, M=M,
                coef_mp=float(np.asarray(inp["coef_mp"]).reshape(-1)[0]),
                coef_sg=float(np.asarray(inp["coef_sg"]).reshape(-1)[0]),
                nw=len(wnames), wslot=wslot, nb=len(bnames), bslot=bslot)
    return in_maps, meta


# ------------------------------------------------------------------ device

def _build(meta, use_ag=False):
    from concourse import bass, bacc, mybir, tile
    nc = bacc.Bacc("TRN2", target_bir_lowering=False, debug=False,
                   num_devices=NCORES)
    f32 = mybir.dt.float32
    bf16 = mybir.dt.bfloat16
    i32 = mybir.dt.int32
    SILU = mybir.ActivationFunctionType.Silu
    EQ = mybir.AluOpType.is_equal
    MULT = mybir.AluOpType.mult
    ADD = mybir.AluOpType.add
    PE_, PT_, PA_ = meta["PE"], meta["PT"], meta["PA"]
    NAW, NES, NTS = meta["NAW"], meta["NES"], meta["NTS"]
    S_aw, aw_super0 = meta["S_aw"], meta["aw_super0"]
    S_tw, tw_super0 = meta["S_tw"], meta["tw_super0"]
    NW, NBI = meta["nw"], meta["nb"]
    wslot, bslot = meta["wslot"], meta["bslot"]
    H = PE_ // 2

    din = {}
    for name, shape, dt in [
        ("rbfT", [NR, PE_], bf16), ("x0iT", [E, PE_], bf16),
        ("x0jT", [E, PE_], bf16), ("asid", [128, PE_ // 128], f32),
        ("gidx", [128, PT_ // 128], i32), ("tsid", [128, PT_ // 128], f32),
        ("abfT", [NABF, PT_], bf16), ("s_init", [E, PA_], f32),
        ("msid", [128, PA_ // 128], f32), ("wpack", [128, NW * 128], bf16),
        ("bpack", [128, NBI], f32), ("iota", [128, 512], f32),
        ("ident", [128, 128], bf16),
    ]:
        din[name] = nc.dram_tensor(name, shape, dt, kind="ExternalInput")
    dout = nc.dram_tensor("out_mol", [128, E], f32, kind="ExternalOutput")

    x_cur = nc.dram_tensor("x_cur", [E, PE_], bf16, kind="Internal")
    x_ji_h = nc.dram_tensor("x_ji_h", [E, PE_], bf16, kind="Internal")
    xkj_loc = nc.dram_tensor("xkj_loc", [PE_, E], bf16, kind="Internal")
    a2a_in0 = nc.dram_tensor("a2a_in0", [NCORES * H, E], bf16, kind="Internal")
    a2a_in1 = nc.dram_tensor("a2a_in1", [NCORES * H, E], bf16, kind="Internal")
    xkj_full = nc.dram_tensor("xkj_full", [2 * NCORES * H, E], bf16,
                              kind="Internal")

    with tile.TileContext(nc) as tc:
        with tc.tile_pool(name="res", bufs=1) as rp, \
             tc.tile_pool(name="wk", bufs=3) as wk, \
             tc.tile_pool(name="gt", bufs=6) as gp, \
             tc.tile_pool(name="pw", bufs=2, space="PSUM") as pw, \
             tc.tile_pool(name="pa", bufs=1, space="PSUM") as pa, \
             tc.tile_pool(name="pm", bufs=2, space="PSUM") as pm, \
             tc.tile_pool(name="px", bufs=2, space="PSUM") as px, \
             tc.tile_pool(name="ptr", bufs=1, space="PSUM") as ptr_p:

            wpack = rp.tile([128, NW * 128], bf16)
            nc.sync.dma_start(out=wpack[:], in_=din["wpack"][:])
            bpack = rp.tile([128, NBI], f32)
            nc.sync.dma_start(out=bpack[:], in_=din["bpack"][:])
            iota = rp.tile([128, 512], f32)
            nc.sync.dma_start(out=iota[:], in_=din["iota"][:])
            ident = rp.tile([128, 128], bf16)
            nc.sync.dma_start(out=ident[:], in_=din["ident"][:])
            asid = rp.tile([128, PE_ // 128], f32)
            nc.sync.dma_start(out=asid[:], in_=din["asid"][:])
            gidx = rp.tile([128, PT_ // 128], i32)
            nc.sync.dma_start(out=gidx[:], in_=din["gidx"][:])
            tsid = rp.tile([128, PT_ // 128], f32)
            nc.sync.dma_start(out=tsid[:], in_=din["tsid"][:])
            msid = rp.tile([128, PA_ // 128], f32)
            nc.sync.dma_start(out=msid[:], in_=din["msid"][:])
            per_atom = rp.tile([128, PA_], f32)
            res_out = rp.tile([128, PA_], f32)
            s_t = rp.tile([128, PA_], f32)
            nc.sync.dma_start(out=s_t[:], in_=din["s_init"][:])
            nc.vector.memset(res_out[:], 0.0)

            def W(n):
                k = wslot[n]
                return wpack[:, k * 128:(k + 1) * 128]

            def Wk(n, K):
                k = wslot[n]
                return wpack[:, k * 128:(k + 1) * 128][0:K, :]

            def Bv(n):
                return bpack[:, bslot[n]:bslot[n] + 1]

            # -------- helper: transpose 4 sub-tiles of a [128,512] bf16
            def transp4(src):
                rows = []
                for cq in range(4):
                    pt_ = ptr_p.tile([128, 128], bf16, tag="tp")
                    nc.tensor.transpose(out=pt_[:], in_=src[:, cq * 128:(cq + 1) * 128],
                                        identity=ident[:])
                    row = wk.tile([128, 128], bf16, tag="tr")
                    nc.vector.tensor_copy(out=row[:], in_=pt_[:])
                    rows.append(row)
                return rows

            # -------- per-atom aggregation for one edge super
            def atom_agg(g_t, s):
                aw = None
                # find atom window of super s
                for a in range(NAW):
                    if aw_super0[a] <= s < aw_super0[a + 1]:
                        aw = a
                        break
                first = (s == aw_super0[aw])
                last = (s == aw_super0[aw + 1] - 1)
                rows = transp4(g_t)
                if first:
                    pa_t = pa.tile([128, 512], f32, tag="pa")
                    atom_agg.cur = pa_t
                pa_t = atom_agg.cur
                for cq in range(4):
                    mk = wk.tile([128, 512], bf16, tag="amask")
                    nc.vector.tensor_tensor(
                        out=mk[:],
                        in0=asid[:, s * 4 + cq:s * 4 + cq + 1].to_broadcast([128, 512]),
                        in1=iota[:], op=EQ)
                    nc.tensor.matmul(out=pa_t[:], lhsT=rows[cq][:], rhs=mk[:],
                                     start=(first and cq == 0),
                                     stop=(last and cq == 3))
                if last:
                    nc.vector.tensor_tensor(
                        out=per_atom[:, aw * 512:(aw + 1) * 512],
                        in0=per_atom[:, aw * 512:(aw + 1) * 512],
                        in1=pa_t[:], op=ADD)

            # -------- output block tail: per_atom -> res_out
            def ob_tail(i):
                for t in range(PA_ // 512):
                    sl = slice(t * 512, (t + 1) * 512)
                    pa_bf = wk.tile([128, 512], bf16, tag="pabf")
                    nc.vector.tensor_copy(out=pa_bf[:], in_=per_atom[:, sl])
                    p1 = pm.tile([128, 512], f32, tag="pm")
                    nc.tensor.matmul(out=p1[:], lhsT=W(f"o1_{i}"), rhs=pa_bf[:],
                                     start=True, stop=True)
                    h = wk.tile([128, 512], bf16, tag="hob")
                    nc.scalar.activation(out=h[:], in_=p1[:], func=SILU,
                                         bias=Bv(f"o1_{i}"))
                    p2 = pm.tile([128, 512], f32, tag="pm")
                    nc.tensor.matmul(out=p2[:], lhsT=W(f"out{i}"), rhs=h[:],
                                     start=True, stop=True)
                    nc.vector.tensor_tensor(out=res_out[:, sl],
                                            in0=res_out[:, sl], in1=p2[:],
                                            op=ADD)

            def identity_block(i, overwrite=False):
                for t in range(PA_ // 512):
                    sl = slice(t * 512, (t + 1) * 512)
                    s_bf = wk.tile([128, 512], bf16, tag="sbf")
                    nc.vector.tensor_copy(out=s_bf[:], in_=s_t[:, sl])
                    p1 = pm.tile([128, 512], f32, tag="pm")
                    nc.tensor.matmul(out=p1[:], lhsT=W(f"id1_{i}"), rhs=s_bf[:],
                                     start=True, stop=True)
                    h = wk.tile([128, 512], bf16, tag="hid")
                    nc.scalar.activation(out=h[:], in_=p1[:], func=SILU,
                                         bias=Bv(f"id1_{i}"))
                    p2 = pm.tile([128, 512], f32, tag="pm")
                    nc.tensor.matmul(out=p2[:], lhsT=W(f"id2_{i}"), rhs=h[:],
                                     start=True, stop=True)
                    if overwrite:
                        nc.vector.tensor_copy(out=s_t[:, sl], in_=p2[:])
                    else:
                        nc.vector.tensor_tensor(out=s_t[:, sl], in0=s_t[:, sl],
                                                in1=p2[:], op=ADD)

            # ================= embedding + output block 0
            nc.vector.memset(per_atom[:], 0.0)
            for s in range(NES):
                sl = slice(s * 512, (s + 1) * 512)
                xi = wk.tile([128, 512], bf16, tag="xi")
                nc.sync.dma_start(out=xi[:], in_=din["x0iT"][:, sl])
                xj = wk.tile([128, 512], bf16, tag="xj")
                nc.sync.dma_start(out=xj[:], in_=din["x0jT"][:, sl])
                rb = wk.tile([NR, 512], bf16, tag="rb")
                nc.sync.dma_start(out=rb[:], in_=din["rbfT"][:, sl])
                p1 = pm.tile([128, 512], f32, tag="pm")
                nc.tensor.matmul(out=p1[:], lhsT=Wk("rbf_emb", NR), rhs=rb[:],
                                 start=True, stop=True)
                rbe = wk.tile([128, 512], bf16, tag="rbe")
                nc.scalar.activation(out=rbe[:], in_=p1[:], func=SILU)
                p2 = pm.tile([128, 512], f32, tag="pm")
                nc.tensor.matmul(out=p2[:], lhsT=W("emb0"), rhs=xi[:],
                                 start=True, stop=False)
                nc.tensor.matmul(out=p2[:], lhsT=W("emb1"), rhs=xj[:],
                                 start=False, stop=False)
                nc.tensor.matmul(out=p2[:], lhsT=W("emb2"), rhs=rbe[:],
                                 start=False, stop=True)
                x_t = wk.tile([128, 512], bf16, tag="xt")
                nc.scalar.activation(out=x_t[:], in_=p2[:], func=SILU,
                                     bias=Bv("emb"))
                nc.sync.dma_start(out=x_cur[:, sl], in_=x_t[:])
                # output block 0 edge part
                p3 = pm.tile([128, 512], f32, tag="pm")
                nc.tensor.matmul(out=p3[:], lhsT=Wk("orbf0", NR), rhs=rb[:],
                                 start=True, stop=True)
                p3b = wk.tile([128, 512], bf16, tag="p3b")
                nc.scalar.copy(out=p3b[:], in_=p3[:])
                g_t = wk.tile([128, 512], bf16, tag="gt")
                nc.vector.tensor_tensor(out=g_t[:], in0=x_t[:], in1=p3b[:],
                                        op=MULT)
                atom_agg(g_t, s)
            ob_tail(0)
            identity_block(0, overwrite=True)

            # ================= interaction blocks
            for i in range(NB):
                # ---- edge pass A: x_ji, x_kj', stores
                for s in range(NES):
                    sl = slice(s * 512, (s + 1) * 512)
                    x_t = wk.tile([128, 512], bf16, tag="xt")
                    nc.sync.dma_start(out=x_t[:], in_=x_cur[:, sl])
                    rb = wk.tile([NR, 512], bf16, tag="rb")
                    nc.sync.dma_start(out=rb[:], in_=din["rbfT"][:, sl])
                    p1 = pm.tile([128, 512], f32, tag="pm")
                    nc.tensor.matmul(out=p1[:], lhsT=W(f"ji{i}"), rhs=x_t[:],
                                     start=True, stop=True)
                    xji = wk.tile([128, 512], bf16, tag="xji")
                    nc.scalar.activation(out=xji[:], in_=p1[:], func=SILU,
                                         bias=Bv(f"ji{i}"))
                    nc.sync.dma_start(out=x_ji_h[:, sl], in_=xji[:])
                    p2 = pm.tile([128, 512], f32, tag="pm")
                    nc.tensor.matmul(out=p2[:], lhsT=W(f"kj{i}"), rhs=x_t[:],
                                     start=True, stop=True)
                    t1 = wk.tile([128, 512], bf16, tag="t1")
                    nc.scalar.activation(out=t1[:], in_=p2[:], func=SILU,
                                         bias=Bv(f"kj{i}"))
                    p3 = pm.tile([128, 512], f32, tag="pm")
                    nc.tensor.matmul(out=p3[:], lhsT=Wk(f"irbf{i}", NR),
                                     rhs=rb[:], start=True, stop=True)
                    p3b = wk.tile([128, 512], bf16, tag="p3b")
                    nc.scalar.copy(out=p3b[:], in_=p3[:])
                    xkj = wk.tile([128, 512], bf16, tag="xkj")
                    nc.vector.tensor_tensor(out=xkj[:], in0=t1[:], in1=p3b[:],
                                            op=MULT)
                    rows = transp4(xkj)
                    for cq in range(4):
                        e0 = s * 512 + cq * 128
                        nc.sync.dma_start(out=xkj_loc[e0:e0 + 128, :],
                                          in_=rows[cq][:])
                # ---- exchange: A2A-as-allgather, two halves
                for h in range(2):
                    buf = a2a_in0 if h == 0 else a2a_in1
                    for dcore in range(NCORES):
                        nc.gpsimd.dma_start(
                            out=buf[dcore * H:(dcore + 1) * H, :],
                            in_=xkj_loc[h * H:(h + 1) * H, :])
                for h in range(2):
                    buf = a2a_in0 if h == 0 else a2a_in1
                    nc.gpsimd.collective_compute(
                        "AllToAll", mybir.AluOpType.bypass,
                        replica_groups=[list(range(NCORES))],
                        ins=[buf[:].opt()],
                        outs=[xkj_full[h * NCORES * H:(h + 1) * NCORES * H, :].opt()],
                    )

                # ---- fused triplet + edge pass B + output block i+1
                nc.vector.memset(per_atom[:], 0.0)
                for w in range(NES):
                    # triplet supers of window w -> PSUM [128, 512]
                    pw_t = pw.tile([128, 512], f32, tag="pw")
                    ns = S_tw[w]
                    s0 = tw_super0[w]
                    for q in range(ns):
                        s = s0 + q
                        ab = wk.tile([NABF, 512], bf16, tag="ab")
                        nc.sync.dma_start(out=ab[:],
                                          in_=din["abfT"][:, s * 512:(s + 1) * 512])
                        g4 = []
                        for cq in range(4):
                            col = s * 4 + cq
                            gg = gp.tile([128, 128], bf16, tag="gg")
                            nc.gpsimd.indirect_dma_start(
                                out=gg[:], out_offset=None,
                                in_=xkj_full[:],
                                in_offset=bass.IndirectOffsetOnAxis(
                                    ap=gidx[:, col:col + 1], axis=0))
                            g4.append(gg)
                        pA = px.tile([128, 512], f32, tag="pA")
                        for cq in range(4):
                            nc.tensor.matmul(
                                out=pA[:, cq * 128:(cq + 1) * 128],
                                lhsT=ab[:, cq * 128:(cq + 1) * 128][0:NABF, :],
                                rhs=W(f"abf{i}")[0:NABF, :],
                                start=True, stop=True)
                        A_bf = wk.tile([128, 512], bf16, tag="Abf")
                        nc.scalar.copy(out=A_bf[:], in_=pA[:])
                        for cq in range(4):
                            m_t = wk.tile([128, 128], bf16, tag="mt")
                            nc.vector.tensor_tensor(
                                out=m_t[:], in0=g4[cq][:],
                                in1=A_bf[:, cq * 128:(cq + 1) * 128], op=MULT)
                            mk = wk.tile([128, 512], bf16, tag="tmask")
                            nc.vector.tensor_tensor(
                                out=mk[:],
                                in0=tsid[:, s * 4 + cq:s * 4 + cq + 1].to_broadcast([128, 512]),
                                in1=iota[:], op=EQ)
                            nc.tensor.matmul(out=pw_t[:], lhsT=m_t[:], rhs=mk[:],
                                             start=(q == 0 and cq == 0),
                                             stop=(q == ns - 1 and cq == 3))
                    # edge pass B on window w
                    sl = slice(w * 512, (w + 1) * 512)
                    xji = wk.tile([128, 512], bf16, tag="xji")
                    nc.sync.dma_start(out=xji[:], in_=x_ji_h[:, sl])
                    pwb = wk.tile([128, 512], bf16, tag="pwb")
                    nc.scalar.copy(out=pwb[:], in_=pw_t[:])
                    h_t = wk.tile([128, 512], bf16, tag="ht")
                    nc.vector.tensor_tensor(out=h_t[:], in0=xji[:], in1=pwb[:],
                                            op=ADD)
                    p1 = pm.tile([128, 512], f32, tag="pm")
                    nc.tensor.matmul(out=p1[:], lhsT=W(f"res{i}"), rhs=h_t[:],
                                     start=True, stop=True)
                    rr = wk.tile([128, 512], bf16, tag="rr")
                    nc.scalar.activation(out=rr[:], in_=p1[:], func=SILU,
                                         bias=Bv(f"res{i}"))
                    x_t = wk.tile([128, 512], bf16, tag="xt")
                    nc.vector.tensor_tensor(out=x_t[:], in0=h_t[:], in1=rr[:],
                                            op=ADD)
                    nc.sync.dma_start(out=x_cur[:, sl], in_=x_t[:])
                    # output block i+1 edge part
                    rb = wk.tile([NR, 512], bf16, tag="rb")
                    nc.sync.dma_start(out=rb[:], in_=din["rbfT"][:, sl])
                    p3 = pm.tile([128, 512], f32, tag="pm")
                    nc.tensor.matmul(out=p3[:], lhsT=Wk(f"orbf{i+1}", NR),
                                     rhs=rb[:], start=True, stop=True)
                    p3b = wk.tile([128, 512], bf16, tag="p3b")
                    nc.scalar.copy(out=p3b[:], in_=p3[:])
                    g_t = wk.tile([128, 512], bf16, tag="gt")
                    nc.vector.tensor_tensor(out=g_t[:], in0=x_t[:], in1=p3b[:],
                                            op=MULT)
                    atom_agg(g_t, w)
                ob_tail(i + 1)
                identity_block(i + 1)

            # ================= final molecule aggregation
            pmol = pa.tile([128, 128], f32, tag="pa")
            nt = PA_ // 128
            for t in range(nt):
                sl = slice(t * 128, (t + 1) * 128)
                tmp = wk.tile([128, 128], f32, tag="tmpf")
                nc.scalar.mul(out=tmp[:], in_=res_out[:, sl],
                              mul=float(meta["coef_mp"]))
                tmp2 = wk.tile([128, 128], f32, tag="tmpf2")
                nc.scalar.mul(out=tmp2[:], in_=s_t[:, sl],
                              mul=float(meta["coef_sg"]))
                totf = wk.tile([128, 128], f32, tag="totf")
                nc.vector.tensor_tensor(out=totf[:], in0=tmp[:], in1=tmp2[:],
                                        op=ADD)
                tot = wk.tile([128, 128], bf16, tag="tot")
                nc.vector.tensor_copy(out=tot[:], in_=totf[:])
                ptr = ptr_p.tile([128, 128], bf16, tag="tp")
                nc.tensor.transpose(out=ptr[:], in_=tot[:], identity=ident[:])
                trow = wk.tile([128, 128], bf16, tag="trow")
                nc.vector.tensor_copy(out=trow[:], in_=ptr[:])
                mk = wk.tile([128, 128], bf16, tag="mmask")
                nc.vector.tensor_tensor(
                    out=mk[:],
                    in0=msid[:, t:t + 1].to_broadcast([128, 128]),
                    in1=iota[:, 0:128], op=EQ)
                nc.tensor.matmul(out=pmol[:], lhsT=mk[:], rhs=trow[:],
                                 start=(t == 0), stop=(t == nt - 1))
            outb = wk.tile([128, 128], f32, tag="outb")
            nc.vector.tensor_copy(out=outb[:], in_=pmol[:])
            nc.sync.dma_start(out=dout[:], in_=outb[:])

    nc.finalize()
    return nc


# ------------------------------------------------------------------ entry

LAST_EXEC_NS = None


def _run_spmd(nc, in_maps, n_reps=8):
    """Execute on 8 cores with device-resident inputs; time repeat runs."""
    import time
    import jax
    from jax.sharding import Mesh, PartitionSpec, NamedSharding
    from jax.experimental.shard_map import shard_map
    from concourse import mybir
    from concourse.bass2jax import (
        _bass_exec_p, install_neuronx_cc_hook, partition_id_tensor)

    install_neuronx_cc_hook()
    partition_name = (nc.partition_id_tensor.name
                      if nc.partition_id_tensor else None)
    in_names, out_names, out_avals, zero_outs = [], [], [], []
    for alloc in nc.m.functions[0].allocations:
        if not isinstance(alloc, mybir.MemoryLocationSet):
            continue
        name = alloc.memorylocations[0].name
        if alloc.kind == "ExternalInput":
            if name != partition_name:
                in_names.append(name)
        elif alloc.kind == "ExternalOutput":
            out_names.append(name)
            shape = tuple(alloc.tensor_shape)
            dtype = mybir.dt.np(alloc.dtype)
            out_avals.append(jax.core.ShapedArray(shape, dtype))
            zero_outs.append(np.zeros(shape, dtype))
    all_in_names = list(in_names) + list(out_names)
    if partition_name is not None:
        all_in_names.append(partition_name)

    def _body(*args):
        operands = list(args)
        if partition_name is not None:
            operands.append(partition_id_tensor())
        outs = _bass_exec_p.bind(
            *operands, out_avals=tuple(out_avals),
            in_names=tuple(all_in_names), out_names=tuple(out_names),
            lowering_input_output_aliases=(),
            sim_require_finite=True, sim_require_nnan=True, nc=nc)
        return tuple(outs)

    devices = jax.devices()[:NCORES]
    mesh = Mesh(np.asarray(devices), ("core",))
    nin = len(in_names) + len(zero_outs)
    fn = jax.jit(shard_map(_body, mesh=mesh,
                           in_specs=(PartitionSpec("core"),) * nin,
                           out_specs=(PartitionSpec("core"),) * len(out_names),
                           check_rep=False), keep_unused=True)
    sharding = NamedSharding(mesh, PartitionSpec("core"))
    args = []
    for name in in_names:
        cat = np.concatenate([np.asarray(in_maps[c][name])
                              for c in range(NCORES)], axis=0)
        args.append(jax.device_put(cat, sharding))
    for z in zero_outs:
        cat = np.zeros((NCORES * z.shape[0], *z.shape[1:]), z.dtype)
        args.append(jax.device_put(cat, sharding))
    jax.block_until_ready(args)

    outs = fn(*args)
    jax.block_until_ready(outs)
    times = []
    for _ in range(n_reps):
        t0 = time.perf_counter()
        o = fn(*args)
        jax.block_until_ready(o)
        times.append(time.perf_counter() - t0)
    global LAST_EXEC_NS
    LAST_EXEC_NS = int(min(times) * 1e9) if times else None
    results = []
    for c in range(NCORES):
        d = {}
        for i, name in enumerate(out_names):
            shape = out_avals[i].shape
            d[name] = np.asarray(outs[i]).reshape(NCORES, *shape)[c]
        results.append(d)
    return results


def kernel(**inputs):
    in_maps, meta = _prep(inputs)
    nc = _build(meta)
    results = _run_spmd(nc, in_maps)
    M = meta["M"]
    out = np.zeros((M, E), np.float32)
    for c in range(NCORES):
        mc = meta["m_hi"][c] - meta["m_lo"][c]
        out[meta["m_lo"][c]:meta["m_hi"][c]] = \
            results[c]["out_mol"][:mc, :]
    return out
